# revision 83
# baseline (speedup 1.0000x reference)
"""Trainium2 Bass kernel for a pre-norm transformer block with banded
(sliding-window) attention.

Reference computation (B=4, T=2048, D=512, H=8 heads, head_dim=64,
FFN=2048, fp32):
    xn = rmsnorm(x) ; qkv = xn @ w_qkv ; banded attention (|q-k| <= 64)
    x  = x + attn_out @ w_out + b_out
    h  = gelu(rmsnorm(x) @ w1 + b1) ; out = x + h @ w2 + b2

Sharding: token-parallel over 8 NeuronCores.  B*T = 8192 tokens -> 1024
tokens per core (each core takes half of one batch row).  Because the
attention is banded with context <= 64, each shard only needs a
64-token halo on each side; row edges are zero-padded and masked.  No
collectives.

Per-core design:
  - activations token-major [128 tokens, D] for norms (free-dim
    reductions), transposed to feature-major via PE transposes for use
    as matmul operands;
  - qkv and both FFN matmuls run in fp8e4 with DoubleRow perf mode
    (weights x16-scaled into fp8 range on the host, rescaled via the
    activation `scale`); everything else bf16 with fp32 PSUM;
  - attention scores are computed TRANSPOSED (S.T[k, q]), k-block-major:
    for key block j the 256-token query window [128j-64, 128j+192) is
    streamed once per head against a SHARED full-K=128 kf stationary --
    q is stored twice (qfA/qfB) with complementary head halves zeroed
    so each head's matmul is a legal (0,0) singleton.  exp(S.T) then
    feeds the AV matmul directly as the moving operand with V
    stationary -- no probability transposes at all;
  - the band mask is multiplicative 0/1, applied on GpSimd after the
    exp (a (64,0)-tile_position matmul cannot open an accumulation
    group, which rules out the additive PE mask for odd heads);
  - softmax row-sums (a partition-dim reduction in this layout) come
    from an all-ones [128k, 64] stationary matmul streamed with P.T --
    the PE broadcasts the sum across output partitions for free;
    normalization is a reciprocal + multiply on VectorE fused with the
    PSUM->SBUF move of the AV result; b_out rides the out-proj matmul
    as a contraction-1 all-ones-row term;
  - rmsnorm: x^2 and its row-sum fused in one ScalarE op (Square +
    accum_out); normalize+gain fused in one VectorE scalar_tensor_tensor;
  - broadcast constants are sent pre-broadcast [128, .] from the host
    (per-partition DMA re-reads of a 1-D vector are ~100x inefficient),
    x tiles round-robin the sync/gpsimd queues ahead of all weight
    loads, and the PE is warmed with dummy transposes so the first qkv
    matmuls hit the fast HAM clock.
"""

import numpy as np
import copy as _copy

import concourse.bass as bass
import concourse.tile as tile
from concourse import mybir
from concourse.bass_utils import run_bass_kernel_spmd

F32 = mybir.dt.float32
BF16 = mybir.dt.bfloat16
FP8 = mybir.dt.float8e4
DR = mybir.MatmulPerfMode.DoubleRow
AX = mybir.AxisListType.X
AF = mybir.ActivationFunctionType
MUL = mybir.AluOpType.mult
PSUM = bass.MemorySpace.PSUM

B, T, D = 4, 2048, 512
H, HD = 8, 64
FFN = 2048
EPS = 1e-6
TPC = 1024          # tokens per core
HALO = 64
TPAD = TPC + 2 * HALO   # 1152 padded tokens
NB = TPAD // 128        # 9 padded token blocks
NQ = TPC // 128         # 8 query blocks


def _bcast_ap(src_1d, p=128):
    """DMA access pattern broadcasting a 1-D DRAM vector across p partitions."""
    return bass.AP(tensor=src_1d.tensor, offset=src_1d.offset,
                   ap=[[0, p]] + [list(a) for a in src_1d.ap])


def _split_waits(nc, maxw=1):
    """Stock walrus rejects instructions carrying more than `maxw` semaphore
    waits; move extras onto preceding no-ops on the same engine."""
    m = nc.m
    new_module = _copy.replace(m, functions=[])
    for function in m.functions:
        new_function = _copy.replace(function, blocks=[])
        new_function.set_allocations_from_list(function.allocations)
        for block in function.blocks:
            insts = []
            for inst in block.instructions:
                si = inst.sync_info
                if si is not None and len(si.on_wait) > maxw:
                    waits = list(si.on_wait)
                    extra, keep = waits[maxw:], waits[:maxw]
                    for j in range(0, len(extra), maxw):
                        insts.append(mybir.InstNoOp(
                            name=f"{inst.name}_wsplit{j}",
                            engine=inst.engine,
                            sync_info=mybir.SyncInfo(
                                on_wait=extra[j:j + maxw], on_update=[]),
                            bass_nofuse=True,
                        ))
                    inst.sync_info = mybir.SyncInfo(
                        on_wait=keep, on_update=list(si.on_update))
                insts.append(inst)
            new_function.blocks.append(_copy.replace(block, instructions=insts))
        new_module.functions.append(new_function)
    nc.m = new_module


def _build_nc():
    nc = bass.Bass("TRN2", debug=False)

    xpad_d = nc.dram_tensor("xpad", [TPAD, D], F32, kind="ExternalInput")
    masks_d = nc.dram_tensor("masks", [3, 128, 512], BF16, kind="ExternalInput")
    wqkv_d = nc.dram_tensor("w_qkv", [128, 2, 2, 3 * D], FP8,
                            kind="ExternalInput")
    wout_d = nc.dram_tensor("w_out", [D, D], BF16, kind="ExternalInput")
    bout_d = nc.dram_tensor("b_out", [128, D], F32, kind="ExternalInput")
    # ffn weights pre-scaled x16 into fp8 range on the host, laid out for
    # DoubleRow: [partition, k-group, 2-interleave, out]
    w1_d = nc.dram_tensor("w1", [128, 2, 2, FFN], FP8, kind="ExternalInput")
    b1_d = nc.dram_tensor("b1", [128, FFN // 128], F32, kind="ExternalInput")
    w2_d = nc.dram_tensor("w2", [128, 8, 2, D], FP8, kind="ExternalInput")
    b2_d = nc.dram_tensor("b2", [128, D], F32, kind="ExternalInput")
    n1_d = nc.dram_tensor("norm1_w", [128, D], F32, kind="ExternalInput")
    n2_d = nc.dram_tensor("norm2_w", [128, D], F32, kind="ExternalInput")
    id_d = nc.dram_tensor("ident", [128, 128], BF16, kind="ExternalInput")
    out_d = nc.dram_tensor("out", [TPC, D], F32, kind="ExternalOutput")

    with tile.TileContext(nc) as tc:
        with (
            tc.tile_pool(name="consts", bufs=1) as consts,
            tc.tile_pool(name="weights", bufs=1) as wpool,
            tc.tile_pool(name="acts", bufs=1) as acts,
            tc.tile_pool(name="xt", bufs=8) as xtp,
            tc.tile_pool(name="scr", bufs=8) as scr,
            tc.tile_pool(name="small", bufs=16) as small,
            tc.tile_pool(name="pt", bufs=2) as ptp,
            tc.tile_pool(name="psum_mm", bufs=2, space=PSUM) as psum_mm,
            tc.tile_pool(name="psum_sT", bufs=2, space=PSUM) as psum_sT,
            tc.tile_pool(name="psum_tr", bufs=2, space=PSUM) as psum_tr,
            tc.tile_pool(name="psum_av", bufs=2, space=PSUM) as psum_av,
        ):
            # ---- constants + input loads -----------------------------------
            # x tiles round-robin over three otherwise-idle DMA queues (sync /
            # gpsimd / tensor) with nothing queued ahead of them; broadcast
            # constants ride the scalar queue; big memsets go to VectorE
            # where they stay off every DMA queue.
            x0 = xtp.tile([128, D], F32, tag="x")
            nc.sync.dma_start(x0[:], xpad_d[0:128, :])
            idt = consts.tile([128, 128], BF16)
            nc.sync.dma_start(idt[:], id_d[:])
            eps_t = consts.tile([128, 1], F32, tag="eps")
            nc.vector.memset(eps_t[:], EPS)
            zero_t = consts.tile([128, 1], F32, tag="zero")
            nc.vector.memset(zero_t[:], 0.0)
            ones64 = consts.tile([128, 64], BF16, tag="ones64")
            nc.vector.memset(ones64[:], 1.0)
            ones_row = consts.tile([1, 128], BF16, tag="ones_row")
            nc.vector.memset(ones_row[:], 1.0)
            s16_t = consts.tile([128, 1], F32, tag="s16")
            nc.vector.memset(s16_t[:], 1.0 / 16)
            warm2 = consts.tile([128, 1], F32, tag="warm")
            nc.scalar.activation(warm2[:], zero_t[:], AF.Square, bias=zero_t[:])
            # only n1 is needed early; the other broadcast loads are emitted
            # after phase 1 so their (expensive) trigger instructions never
            # stall the rmsnorm chain or the x-tile queues
            n1_b = consts.tile([128, D], F32, tag="n1")
            nc.scalar.dma_start(n1_b[:], n1_d[:])
            n2_b = consts.tile([128, D], F32, tag="n2")
            bout_b = consts.tile([128, D], F32, tag="bo")
            b2_b = consts.tile([128, D], F32, tag="b2")
            b1_fm = consts.tile([128, FFN // 128], F32, tag="b1")
            m_sb = consts.tile([128, 3, 512], BF16)

            # PE warm-up: dummy transposes keep the PE HAM activity window
            # alive while the first rmsnorm chain runs, so qkv starts at the
            # warm clock
            for wi in range(6):
                ptw = psum_tr.tile([128, 128], BF16, tag="tr", name=f"warmt{wi}")
                nc.tensor.transpose(ptw[:], idt[:], idt[:])

            # ---- weights (already bf16 from host) --------------------------
            # emitted lazily so the x-tile DMAs reach the queue first and
            # later weight loads overlap earlier compute phases
            def load_w(dram, kchunks, ncols, tag, dt=BF16):
                w = wpool.tile([128, kchunks, ncols], dt, tag=tag)
                for c in range(kchunks):
                    nc.sync.dma_start(w[:, c, :], dram[128 * c:128 * (c + 1), :])
                return w

            # ---- phase 1: load x, rmsnorm, transpose to feature-major ------
            def rmsnorm(xt, nw_b, xnb, sq_vec=False):
                """token-major rmsnorm: xnb = xt / rms(xt) * nw (bf16 out).
                sq_vec computes the sum of squares on VectorE instead of
                ScalarE so alternating blocks use different engines."""
                sq = small.tile([128, 1], F32, tag="sq")
                if sq_vec:
                    s = scr.tile([128, D], F32, tag="s")
                    nc.vector.tensor_tensor(s[:], xt[:], xt[:], MUL)
                    nc.vector.reduce_sum(sq[:], s[:], axis=AX)
                else:
                    # the squared tensor itself is discarded (only accum_out
                    # matters) -- bf16 output halves the ScalarE write cost
                    s = scr.tile([128, D], BF16, tag="sb")
                    nc.scalar.activation(s[:], xt[:], AF.Square,
                                         bias=zero_t[:], accum_out=sq[:])
                rms = small.tile([128, 1], F32, tag="rms")
                nc.scalar.activation(rms[:], sq[:], AF.Sqrt, bias=eps_t[:],
                                     scale=1.0 / D)
                inv = small.tile([128, 1], F32, tag="inv")
                nc.vector.reciprocal(inv[:], rms[:])
                if nw_b is None:
                    # gain pre-folded into the consuming weight on the host;
                    # per-partition-scaled copy on ScalarE keeps this off the
                    # saturated VectorE in the attention window
                    nc.scalar.activation(xnb[:], xt[:], AF.Copy, scale=inv[:])
                else:
                    nc.vector.scalar_tensor_tensor(
                        xnb[:], xt[:], inv[:], nw_b[:], op0=MUL, op1=MUL)

            xnT = acts.tile([128, 4, TPAD], FP8, tag="xnT")
            for i in range(NB):
                if i == 0:
                    xt = x0
                else:
                    xt = xtp.tile([128, D], F32, tag="x")
                    eng = nc.sync if i % 2 == 0 else nc.gpsimd
                    eng.dma_start(xt[:], xpad_d[128 * i:128 * (i + 1), :])
                xnb = xtp.tile([128, D], BF16, tag="xnb")
                rmsnorm(xt, n1_b, xnb)
                for c in range(4):
                    pt = psum_tr.tile([128, 128], BF16, tag="tr")
                    nc.tensor.transpose(pt[:], xnb[:, 128 * c:128 * (c + 1)], idt[:])
                    if c % 2 == 0:
                        nc.vector.tensor_copy(xnT[:, c, 128 * i:128 * (i + 1)], pt[:])
                    else:
                        nc.scalar.copy(xnT[:, c, 128 * i:128 * (i + 1)], pt[:])

            wqkv = wpool.tile([128, 2, 2, 3 * D], FP8, tag="wqkv")
            for g in range(2):
                nc.sync.dma_start(wqkv[:, g, :, :], wqkv_d[:, g, :, :])
            nc.gpsimd.dma_start(n2_b[:], n2_d[:])
            nc.gpsimd.dma_start(bout_b[:], bout_d[:])
            nc.gpsimd.dma_start(b2_b[:], b2_d[:])
            nc.gpsimd.dma_start(b1_fm[:], b1_d[:])
            nc.gpsimd.dma_start(m_sb[:], masks_d.rearrange("m p k -> p m k"))

            # ---- phase 2: qkv ---------------------------------------------
            # q, k feature-major [dim, tok]; v token-major [tok, dim].
            # q is stored twice with complementary head halves zeroed (qfA:
            # even head real / odd zero, qfB: the reverse) so each head's
            # banded-score matmul can stream a full-K=128 operand against the
            # SHARED kf stationary -- no mask matmuls, no identity loads, and
            # both heads of a pair reuse one weight load.  64 zero-padded
            # columns on each side let every query window be 256 wide.
            qfA = acts.tile([128, 4, TPAD + 128], BF16, tag="big")
            nc.gpsimd.memset(qfA[64:128, :, :], 0.0)
            nc.vector.memset(qfA[0:64, :, 0:64], 0.0)
            nc.vector.memset(qfA[0:64, :, TPAD + 64:TPAD + 128], 0.0)
            qfB = acts.tile([128, 4, TPAD + 128], BF16, tag="qfB")
            nc.gpsimd.memset(qfB[0:64, :, :], 0.0)
            nc.vector.memset(qfB[64:128, :, 0:64], 0.0)
            nc.vector.memset(qfB[64:128, :, TPAD + 64:TPAD + 128], 0.0)
            kf = acts.tile([128, 4, TPAD], BF16, tag="kf")
            for m in range(8):
                for j in range(3):
                    ps = psum_mm.tile([128, 384], F32, tag="mm")
                    for g in range(2):
                        nc.tensor.matmul(
                            ps[:], wqkv[:, g, :, 128 * m:128 * (m + 1)],
                            xnT[:, 2 * g:2 * g + 2, 384 * j:384 * (j + 1)],
                            start=(g == 0), stop=(g == 1), perf_mode=DR)
                    if m < 4:  # q: fold in 1/sqrt(head_dim) and the /16
                        nc.scalar.activation(
                            qfA[0:64, m, 64 + 384 * j:64 + 384 * (j + 1)],
                            ps[0:64, :], AF.Copy, scale=HD ** -0.5 / 16)
                        nc.vector.tensor_scalar_mul(
                            qfB[64:128, m, 64 + 384 * j:64 + 384 * (j + 1)],
                            ps[64:128, :], HD ** -0.5 / 16)
                    else:
                        nc.vector.tensor_scalar_mul(
                            kf[:, m - 4, 384 * j:384 * (j + 1)], ps[:],
                            1.0 / 16)
            vt = acts.tile([128, NB, D], BF16, tag="vt")

            def v_block(i):
                ps = psum_mm.tile([128, 512], F32, tag="mm")
                for g in range(2):
                    nc.tensor.matmul(ps[:], xnT[:, 2 * g:2 * g + 2,
                                               128 * i:128 * (i + 1)],
                                     wqkv[:, g, :, 1024:1536],
                                     start=(g == 0), stop=(g == 1),
                                     perf_mode=DR)
                nc.vector.tensor_scalar_mul(vt[:, i, :], ps[:], 1.0 / 16)

            wout = load_w(wout_d, 4, D, "wout")
            bout_bf = consts.tile([1, D], BF16, tag="bout_bf")
            nc.scalar.copy(bout_bf[:], bout_b[0:1, :])
            w1b = wpool.tile([128, 2, 2, FFN], FP8, tag="w1")
            for g in range(2):
                nc.sync.dma_start(w1b[:, g, :, :], w1_d[:, g, :, :])
            w2b = wpool.tile([128, 8, 2, D], FP8, tag="w2")
            for g in range(8):
                nc.sync.dma_start(w2b[:, g, :, :], w2_d[:, g, :, :])

            # ---- phase 3: banded attention, transposed scores --------------
            # Per key block j and head pair: S.T[k, q] over the 256-wide query
            # window, band mask accumulated first.  exp on ScalarE straight
            # from PSUM -> P.T in SBUF.  Per query block: AV (V stationary,
            # P.T streaming) and row-sums (ones stationary) on the PE;
            # normalize on VectorE fused with the PSUM->SBUF move.
            attn_f = acts.tile([128, 4, TPC], BF16, tag="a2")
            pt_tiles = {}

            def scores_block(j):
                msel = 0 if j == 0 else (2 if j == NB - 1 else 1)
                pt = ptp.tile([128, 4, 2, 256], BF16, tag="pt", name=f"pt{j}")
                pt_tiles[j] = pt
                for hp in range(4):
                    ps = psum_sT.tile([128, 2, 256], F32, tag="sT")
                    # both heads of the pair stream against the SAME kf
                    # stationary (full K=128); the zeroed half of qfA/qfB
                    # keeps the other head's contribution out
                    nc.tensor.matmul(
                        ps[:, 0, :], kf[:, hp, 128 * j:128 * (j + 1)],
                        qfA[:, hp, 128 * j:128 * j + 256],
                        start=True, stop=True)
                    nc.tensor.matmul(
                        ps[:, 1, :], kf[:, hp, 128 * j:128 * (j + 1)],
                        qfB[:, hp, 128 * j:128 * j + 256],
                        start=True, stop=True)
                    ptE = scr.tile([128, 2, 256], BF16, tag="ptE")
                    nc.scalar.activation(ptE[:], ps[:], AF.Exp, bias=zero_t[:])
                    # multiplicative 0/1 band mask (duplicated per head
                    # half) on the otherwise-idle GpSimd
                    nc.gpsimd.tensor_tensor(pt[:, hp, :, :], ptE[:],
                                            m_sb[:, msel, :], MUL)

            def av_block(qb):
                for hp in range(4):
                    ps2 = psum_av.tile([128, 2, 128], F32, tag="avrs")
                    # complete the row-sum group before starting the AV group:
                    # the accumulation-group state is bank-granular
                    for w in range(2):
                        sl = slice(128, 256) if w == 0 else slice(0, 128)
                        pt = pt_tiles[qb + w]
                        for hi in range(2):
                            nc.tensor.matmul(
                                ps2[64 * hi:64 * hi + 64, 1, :], ones64[:],
                                pt[:, hp, hi, sl],
                                start=(w == 0), stop=(w == 1),
                                skip_group_check=True)
                    for w in range(2):
                        sl = slice(128, 256) if w == 0 else slice(0, 128)
                        pt = pt_tiles[qb + w]
                        for hi in range(2):
                            h = 2 * hp + hi
                            nc.tensor.matmul(
                                ps2[64 * hi:64 * hi + 64, 0, :],
                                vt[:, qb + w, 64 * h:64 * (h + 1)],
                                pt[:, hp, hi, sl],
                                start=(w == 0), stop=(w == 1),
                                skip_group_check=True)
                    rcp = scr.tile([128, 128], F32, tag="rcp")
                    nc.vector.reciprocal(rcp[:], ps2[:, 1, :])
                    nc.vector.tensor_tensor(
                        attn_f[:, hp, 128 * qb:128 * (qb + 1)],
                        ps2[:, 0, :], rcp[:], MUL)

            # ---- phase 4 (interleaved): out-proj + residual + rmsnorm2 ----
            x2_all = acts.tile([128, NQ, D], BF16, tag="x2")
            xn2T = acts.tile([128, 4, TPC], FP8, tag="xn2T")

            def outproj_block(j):
                ps = psum_mm.tile([128, 512], F32, tag="mm")
                for c in range(4):
                    nc.tensor.matmul(ps[:], attn_f[:, c, 128 * j:128 * (j + 1)],
                                     wout[:, c, :], start=(c == 0), stop=False)
                # fold the b_out add into the accumulation group: a
                # contraction-1 matmul with an all-ones stationary row
                nc.tensor.matmul(ps[:], ones_row[:],
                                 bout_bf[:], start=False, stop=True)
                xc = xtp.tile([128, D], F32, tag="x")
                nc.sync.dma_start(xc[:], xpad_d[HALO + 128 * j:HALO + 128 * (j + 1), :])
                x2 = x2_all[:, j, :]
                nc.vector.tensor_add(x2, ps[:], xc[:])
                xn2b = xtp.tile([128, D], BF16, tag="xnb")
                rmsnorm(x2, n2_b, xn2b)
                for c in range(4):
                    pt = psum_tr.tile([128, 128], BF16, tag="tr")
                    nc.tensor.transpose(pt[:], xn2b[:, 128 * c:128 * (c + 1)], idt[:])
                    nc.scalar.copy(xn2T[:, c, 128 * j:128 * (j + 1)], pt[:])

            for j in range(NB):
                v_block(j)
                scores_block(j)
                if j >= 1:
                    av_block(j - 1)
            for j in range(NQ):
                outproj_block(j)

            # ---- phase 5: FFN (fp8 DoubleRow, weights x16 on host) ---------
            # half-major so FFN2 on the first token half overlaps FFN1's
            # second half (gelu of half h gates only that half's FFN2)
            hf = acts.tile([128, 16, TPC], FP8, tag="big")
            for half in range(2):
                for m in range(16):
                    ps0 = psum_mm.tile([128, 512], F32, tag="mm")
                    for g in range(2):
                        nc.tensor.matmul(
                            ps0[:], w1b[:, g, :, 128 * m:128 * (m + 1)],
                            xn2T[:, 2 * g:2 * g + 2, 512 * half:512 * (half + 1)],
                            start=(g == 0), stop=(g == 1), perf_mode=DR)
                    nc.scalar.activation(hf[:, m, 512 * half:512 * (half + 1)],
                                         ps0[:], AF.Gelu,
                                         bias=b1_fm[:, m:m + 1], scale=1.0 / 16)
                for j in range(4 * half, 4 * half + 4):
                    ps = psum_mm.tile([128, 512], F32, tag="mm")
                    for g in range(8):
                        nc.tensor.matmul(ps[:], hf[:, 2 * g:2 * g + 2,
                                                   128 * j:128 * (j + 1)],
                                         w2b[:, g, :, :],
                                         start=(g == 0), stop=(g == 7),
                                         perf_mode=DR)
                    o1 = scr.tile([128, D], F32, tag="s")
                    nc.vector.scalar_tensor_tensor(o1[:], ps[:], s16_t[:],
                                                   b2_b[:], op0=MUL,
                                                   op1=mybir.AluOpType.add)
                    o2 = xtp.tile([128, D], F32, tag="o2")
                    nc.vector.tensor_add(o2[:], o1[:], x2_all[:, j, :])
                    nc.sync.dma_start(out_d[128 * j:128 * (j + 1), :], o2[:])

    nc.finalize()
    if _DO_SPLIT_WAITS:
        _split_waits(nc)
    return nc


_DO_SPLIT_WAITS = True
_NC = None


def _get_nc():
    global _NC
    if _NC is None:
        _NC = _build_nc()
    return _NC


def _make_in_maps(x, norm1_w, norm2_w, w_qkv, w_out, b_out, w1, b1, w2, b2,
                  context_size):
    import ml_dtypes
    bf16 = ml_dtypes.bfloat16
    c = int(np.asarray(context_size))
    assert c <= HALO, f"context_size {c} exceeds compiled halo {HALO}"
    x = np.ascontiguousarray(np.asarray(x, np.float32))
    fp8 = ml_dtypes.float8_e4m3
    # ffn weights: x16 into fp8 range, DoubleRow layout
    # [partition, k-group, 2-interleave, out] with k = 128*(2g+e)+p
    w1_f8 = (np.asarray(w1, np.float32) * 16).reshape(2, 2, 128, FFN) \
        .transpose(2, 0, 1, 3).astype(fp8)
    w2_f8 = (np.asarray(w2, np.float32) * 16).reshape(8, 2, 128, D) \
        .transpose(2, 0, 1, 3).astype(fp8)
    wqkv_f8 = (np.asarray(w_qkv, np.float32) * 16).reshape(2, 2, 128, 3 * D) \
        .transpose(2, 0, 1, 3).astype(fp8)
    shared = {
        "w_qkv": np.ascontiguousarray(wqkv_f8),
        "w_out": np.ascontiguousarray(np.asarray(w_out).astype(bf16)),
        "w1": np.ascontiguousarray(w1_f8),
        "w2": np.ascontiguousarray(w2_f8),
        "b_out": np.ascontiguousarray(np.broadcast_to(
            np.asarray(b_out, np.float32), (128, D))),
        "b1": np.ascontiguousarray(
            np.asarray(b1, np.float32).reshape(FFN // 128, 128).T),
        "b2": np.ascontiguousarray(np.broadcast_to(
            np.asarray(b2, np.float32), (128, D))),
        "norm1_w": np.ascontiguousarray(np.broadcast_to(
            np.asarray(norm1_w, np.float32), (128, D))),
        "norm2_w": np.ascontiguousarray(np.broadcast_to(
            np.asarray(norm2_w, np.float32), (128, D))),
        "ident": np.eye(128, dtype=bf16),
    }
    in_maps = []
    o = np.arange(128)[:, None]   # key offset within block (partition)
    u = np.arange(256)[None, :]   # query offset within 256-wide window
    for core in range(8):
        b, t0 = core // 2, (core % 2) * TPC
        lo, hi = t0 - HALO, t0 + TPC + HALO
        xp = np.zeros((TPAD, D), np.float32)
        s0, s1 = max(lo, 0), min(hi, T)
        xp[s0 - lo:s0 - lo + (s1 - s0)] = x[b, s0:s1]
        # Transposed multiplicative masks, k-block-major: maskT[o, u] guards
        # key 128j + o (partition) against query 128j - 64 + u (padded
        # coords); duplicated along the free dim for the two heads of a pair.
        masks = np.empty((3, 128, 256), np.float32)
        for mi, j in ((0, 0), (1, 3), (2, NB - 1)):
            kg = t0 - HALO + 128 * j + o
            qg = t0 - HALO + 128 * j - 64 + u
            ok = (np.abs(qg - kg) <= c) & (kg >= 0) & (kg < T) \
                & (qg >= 0) & (qg < T)
            masks[mi] = ok.astype(np.float32)
        masks = np.concatenate([masks, masks], axis=2)  # dup for head pairs
        in_maps.append({"xpad": xp, "masks": masks.astype(bf16), **shared})
    return in_maps


def _run(in_maps, **kwargs):
    return run_bass_kernel_spmd(_get_nc(), in_maps, core_ids=list(range(8)),
                                **kwargs)


def kernel(**inputs):
    in_maps = _make_in_maps(**inputs)
    res = _run(in_maps)
    out = np.empty((B, T, D), np.float32)
    for core in range(8):
        b, t0 = core // 2, (core % 2) * TPC
        out[b, t0:t0 + TPC] = res.results[core]["out"]
    return out
